# revision 26
# baseline (speedup 1.0000x reference)
"""Trainium2 Bass kernel for nn_Aux2_46969762349381 (scatter_memory).

Computes, for embs [32, 2048, 1024] f32:
  status_probs = softmax(embs @ W_status.T + b_status)   # [B,T,5]
  flight_probs = softmax(embs @ W_flight.T + b_flight)   # [B,T,30]
  out = concat([s0, s2, s1, s4*flight, s3*flight], -1)   # [B,T,63]

Strategy (pure data parallel over batch, 8 cores; full inputs in, full
output out):
  - each core owns 4 batches = 8192 tokens, token t = p*64 + i
    (p = SBUF partition, i = token-tile index) so both the embs loads and
    the out stores are contiguous >=4KB per partition.
  - embs tiles load naturally [128 tok, 1024 emb]; PE transpose (identity
    matmul) flips 128x128 blocks into PSUM; DVE/ACT copy them to SBUF
    giving embsT [128 emb, 8*512 tok].
  - 8 accumulating matmuls (lhsT = host-pretransposed W [128,35] per
    emb-chunk, rhs = embsT chunk [128, 512], float32r) -> psum [35, 512]
    logits.T per 512-token group.
  - ScalarE exp reads the PSUM logits with the per-partition class bias
    fused into the activation -> expT [35, 512] in SBUF.
  - PE transposes expT back to [128 tok, 35] PSUM; DVE does the softmax
    normalization + outer-product scatter into [128, ntile, 63] which DMAs
    out as >=2KB contiguous chunks per partition.

Default precision: embs are cast f32->bf16 during the SWDGE load DMA and
the two tiny matmuls run in bf16 (abs-max relative error ~1.7e-3 vs the
f32 reference; the small heads keep everything else f32). Set
BASS_KERNEL_PRECISE=1 for an all-f32/f32r build (~1.2e-4, ~30% slower:
PE is_transpose runs 2 cycles/row for f32 vs 1 for bf16 and the PE clock
stays at 1.2 GHz because transpose-mode does not engage the HAM).
"""

import os
import sys

import numpy as np

for _p in ("/opt/trn_rl_repo", "/root/.axon_site/_ro/trn_rl_repo"):
    if os.path.isdir(_p) and _p not in sys.path:
        sys.path.insert(0, _p)

from contextlib import ExitStack

import concourse.bass as bass
import concourse.tile as tile
from concourse import mybir
from concourse.bass_utils import run_bass_kernel_spmd

N_CORES = 8
B, T, E = 32, 2048, 1024
NS, NF = 5, 30
NCLS = NS + NF          # 35 combined classes
OUTC = 63
P = 128                 # SBUF partitions
ECH = E // P            # 8 emb chunks of 128
GT = 4                  # token tiles (of 128 tokens) per matmul group
GTOK = GT * P           # 512 tokens per group
AG = 2                  # groups per assembly batch
F32 = mybir.dt.float32
F32R = mybir.dt.float32r
BF16 = mybir.dt.bfloat16
EXP = mybir.ActivationFunctionType.Exp


def _split_multiwait(nc, max_waits=1):
    """Workaround for this walrus build rejecting >1 sem-wait on one
    instruction: move extra waits onto single-wait NoOps just before it."""
    for bb in nc.m.functions[0].blocks:
        insts = list(bb.instructions)
        new_list = []
        changed = False
        for inst in insts:
            si = inst.sync_info
            if si is not None and si.on_wait and len(si.on_wait) > max_waits:
                waits = list(si.on_wait)
                for w in waits[:-max_waits]:
                    nop = mybir.InstNoOp(
                        name=nc.get_next_instruction_name(),
                        ins=[],
                        outs=[],
                        engine=inst.engine,
                        sync_info=mybir.SyncInfo(on_wait=[w], on_update=[]),
                    )
                    nc.register_instruction(nop)
                    new_list.append(nop)
                    changed = True
                inst.sync_info = mybir.SyncInfo(
                    on_wait=waits[-max_waits:], on_update=list(si.on_update)
                )
            new_list.append(inst)
        if changed:
            bb.instructions = new_list


def build_program(tok, copy_split=4, mm_dtype=F32R, tr_dtype=F32, loop_reps=0,
                  passes=1, lpd=2, emb_bufs=4):
    """Build the per-core Bass program for `tok` tokens (tok % 1024 == 0).

    loop_reps > 0 wraps the whole body in a hardware For_i loop executing it
    that many times — benchmarking only (the axon dispatch overhead is ~80ms,
    so single-shot wall timing can't see the ~100us kernel).
    """
    S = tok // P            # token tiles per core
    n_groups = S // GT
    n_batches = n_groups // AG
    NT = AG * GT            # tiles per assembly batch (8)

    nc = bass.Bass("TRN2", num_devices=N_CORES)
    embs_d = nc.dram_tensor("embs", [tok, E], F32, kind="ExternalInput")
    w_d = nc.dram_tensor("wt", [P, ECH * NCLS], F32, kind="ExternalInput")
    b_d = nc.dram_tensor("bias", [NCLS, 1], F32, kind="ExternalInput")
    id_d = nc.dram_tensor("ident", [P, P], F32, kind="ExternalInput")
    out_d = nc.dram_tensor("out", [tok, OUTC], F32, kind="ExternalOutput")

    with tile.TileContext(nc) as tc, ExitStack() as ctx:
        consts = ctx.enter_context(tc.tile_pool(name="consts", bufs=1))
        emb_pool = ctx.enter_context(tc.tile_pool(name="emb", bufs=emb_bufs))
        embT_pool = ctx.enter_context(tc.tile_pool(name="embT", bufs=2))
        expT_pool = ctx.enter_context(tc.tile_pool(name="expT", bufs=2))
        small = ctx.enter_context(tc.tile_pool(name="small", bufs=2))
        outsb = ctx.enter_context(tc.tile_pool(name="outsb", bufs=2))
        psT_pool = ctx.enter_context(tc.tile_pool(name="psT", bufs=4, space="PSUM"))
        psmm_pool = ctx.enter_context(tc.tile_pool(name="psmm", bufs=2, space="PSUM"))
        psxb_pool = ctx.enter_context(tc.tile_pool(name="psxb", bufs=2, space="PSUM"))

        w_raw = consts.tile([P, ECH * NCLS], F32)
        nc.sync.dma_start(w_raw[:], w_d.ap())
        b_sb = consts.tile([NCLS, 1], F32)
        nc.sync.dma_start(b_sb[:], b_d.ap())
        id_sb = consts.tile([P, P], F32)
        if tr_dtype is F32:
            nc.sync.dma_start(id_sb[:], id_d.ap())
        else:
            nc.sync.dma_start(id_sb[:].bitcast(tr_dtype), id_d.ap().bitcast(tr_dtype))
        w_sb = consts.tile([P, ECH * NCLS], mm_dtype)
        if mm_dtype is F32:
            w_sb = w_raw
        else:
            nc.vector.tensor_copy(w_sb[:], w_raw[:])
        if tr_dtype is BF16:
            id_bf = consts.tile([P, P], BF16)
            nc.vector.tensor_copy(id_bf[:], id_sb[:])

        # Trigger the ACT exp table load (~2.7us) immediately so it overlaps
        # the first embs DMAs instead of stalling the first real exp.
        warm = consts.tile([NCLS, 1], F32)
        nc.scalar.activation(warm[:], b_sb[:], EXP)

        embs_v = embs_d.ap().rearrange("(p i) e -> p i e", p=P, i=S)
        out_v = out_d.ap().rearrange("(p i) c -> p i c", p=P, i=S)

        loop_ctx = tc.For_i(0, loop_reps, 1) if loop_reps else None
        if loop_ctx is not None:
            ctx.enter_context(loop_ctx)

        for ab in range(n_batches * passes):
            ab = ab % n_batches
            ps_xb = psxb_pool.tile([P, NT * NCLS], F32)
            for g2 in range(AG):
                g = ab * AG + g2
                embT = embT_pool.tile([P, ECH * GTOK], mm_dtype)
                embT_v = embT[:].rearrange("p (j t) -> p j t", t=GTOK)
                copy_k = 0
                # rows p*64 + 4g + c are adjacent, so each partition reads
                # lpd*E contiguous f32 per DMA at full rate.
                if tr_dtype is BF16:
                    tr = lambda ap: ap
                    ident = id_bf
                    nat_dt = BF16
                else:
                    tr = (lambda ap: ap.bitcast(tr_dtype)) if tr_dtype is not F32 else (lambda ap: ap)
                    ident = id_sb
                    nat_dt = F32
                chunks = []
                for lc in range(GT // lpd):
                    emb_ch = emb_pool.tile([P, lpd * E], nat_dt)
                    dst3 = emb_ch[:].rearrange("p (c e) -> p c e", e=E)
                    src3 = embs_v[:, g * GT + lc * lpd:g * GT + (lc + 1) * lpd, :]
                    if tr_dtype is BF16:
                        nc.gpsimd.dma_start(dst3, src3)  # SWDGE f32->bf16 cast
                    else:
                        nc.sync.dma_start(tr(dst3), tr(src3))
                    chunks.append(dst3)
                for c in range(GT):
                    nat3 = chunks[c // lpd]
                    for half in range(2):
                        psT = psT_pool.tile([P, 4 * P], nat_dt)
                        for jj in range(4):
                            j = half * 4 + jj
                            nc.tensor.matmul(
                                tr(psT[:, jj * P:(jj + 1) * P]),
                                tr(nat3[:, c % lpd, j * P:(j + 1) * P]),
                                tr(ident[:]),
                                is_transpose=True,
                            )
                        dst = embT_v[:, half * 4:(half + 1) * 4, c * P:(c + 1) * P]
                        src = psT[:].rearrange("p (j t) -> p j t", j=4)
                        if copy_k < copy_split:
                            nc.scalar.copy(dst, src)
                        else:
                            nc.vector.tensor_copy(dst, src)
                        copy_k += 1
                ps_mm = psmm_pool.tile([NCLS, GTOK], F32)
                for j in range(ECH):
                    nc.tensor.matmul(
                        ps_mm[:],
                        w_sb[:, j * NCLS:(j + 1) * NCLS],
                        embT[:, j * GTOK:(j + 1) * GTOK],
                        start=(j == 0),
                        stop=(j == ECH - 1),
                    )
                expT = expT_pool.tile([NCLS, GTOK], F32)
                nc.scalar.activation(expT[:], ps_mm[:], EXP, bias=b_sb[:], scale=1.0)
                for c in range(GT):
                    it = g2 * GT + c
                    nc.tensor.matmul(
                        ps_xb[:, it * NCLS:(it + 1) * NCLS],
                        expT[:, c * P:(c + 1) * P],
                        id_sb[0:NCLS, 0:NCLS],
                        is_transpose=True,
                    )

            # ---- assembly for NT tiles (tokens p*64 + ab*NT .. +NT) ----
            X = ps_xb[:].rearrange("p (i c) -> p i c", c=NCLS)  # [128, NT, 35]
            sums = small.tile([P, 2 * NT], F32)
            nc.vector.reduce_sum(sums[:, 0:NT], X[:, :, 0:NS], axis=mybir.AxisListType.X)
            nc.vector.reduce_sum(
                sums[:, NT:2 * NT], X[:, :, NS:NCLS], axis=mybir.AxisListType.X
            )
            inv = small.tile([P, 2 * NT], F32)
            nc.vector.reciprocal(inv[:], sums[:])
            inv_s = inv[:, 0:NT]
            inv_f = inv[:, NT:2 * NT]
            dd = small.tile([P, NT], F32)
            nc.vector.tensor_mul(dd[:], inv_s, inv_f)
            cc = small.tile([P, 2 * NT], F32)
            nc.vector.tensor_mul(cc[:, 0:NT], X[:, :, 4], dd[:])  # book scale
            nc.vector.tensor_mul(cc[:, NT:2 * NT], X[:, :, 3], dd[:])  # change
            o_sb = outsb.tile([P, NT * OUTC], F32)
            O = o_sb[:].rearrange("p (i c) -> p i c", c=OUTC)
            inv_s3 = inv_s.unsqueeze(2)
            nc.vector.tensor_mul(O[:, :, 0:1], X[:, :, 0:1], inv_s3)
            nc.vector.tensor_mul(O[:, :, 1:2], X[:, :, 2:3], inv_s3)
            nc.vector.tensor_mul(O[:, :, 2:3], X[:, :, 1:2], inv_s3)
            nc.vector.tensor_mul(
                O[:, :, 3:3 + NF],
                X[:, :, NS:NCLS],
                cc[:, 0:NT].unsqueeze(2).broadcast_to((P, NT, NF)),
            )
            nc.vector.tensor_mul(
                O[:, :, 3 + NF:OUTC],
                X[:, :, NS:NCLS],
                cc[:, NT:2 * NT].unsqueeze(2).broadcast_to((P, NT, NF)),
            )
            store_eng = nc.sync if tr_dtype is BF16 else nc.gpsimd
            store_eng.dma_start(out_v[:, ab * NT:(ab + 1) * NT, :], o_sb[:].rearrange("p (i c) -> p i c", c=OUTC))

    _split_multiwait(nc)
    return nc


def host_inputs(W_status, b_status, W_flight, b_flight):
    W = np.concatenate([np.asarray(W_status), np.asarray(W_flight)], axis=0)
    W = np.ascontiguousarray(W, dtype=np.float32)          # [35, 1024]
    # w_host[p, j*35 + c] = W[c, j*128 + p]
    w_host = np.ascontiguousarray(
        W.T.reshape(ECH, P, NCLS).transpose(1, 0, 2).reshape(P, ECH * NCLS)
    )
    b_host = np.ascontiguousarray(
        np.concatenate([np.asarray(b_status), np.asarray(b_flight)]).reshape(NCLS, 1),
        dtype=np.float32,
    )
    ident = np.eye(P, dtype=np.float32)
    return w_host, b_host, ident


_program_cache = {}


def kernel(embs, W_status, b_status, W_flight, b_flight, **run_kwargs):
    embs = np.ascontiguousarray(np.asarray(embs), dtype=np.float32)
    tok = embs.shape[0] * embs.shape[1] // N_CORES
    w_host, b_host, ident = host_inputs(W_status, b_status, W_flight, b_flight)

    precise = bool(os.environ.get("BASS_KERNEL_PRECISE"))
    key = (tok, precise)
    nc = _program_cache.get(key)
    if nc is None:
        if precise:
            # f32r everywhere: rel err ~1.2e-4, ~30% slower (PE transposes)
            nc = build_program(tok, tr_dtype=F32R, lpd=1)
        else:
            # bf16 embs (cast during DMA) + bf16 PE: rel err ~1.7e-3
            nc = build_program(tok, tr_dtype=BF16, mm_dtype=BF16)
        _program_cache[key] = nc

    embs_flat = embs.reshape(-1, E)
    in_maps = [
        {
            "embs": embs_flat[c * tok:(c + 1) * tok],
            "wt": w_host,
            "bias": b_host,
            "ident": ident,
        }
        for c in range(N_CORES)
    ]
    res = run_bass_kernel_spmd(
        nc, in_maps, core_ids=list(range(N_CORES)), **run_kwargs
    )
    out = np.concatenate([res.results[c]["out"] for c in range(N_CORES)], axis=0)
    out = out.reshape(embs.shape[0], embs.shape[1], OUTC)
    if run_kwargs:
        return out, res
    return out


# revision 30
# speedup vs baseline: 1.1996x; 1.1996x over previous
"""Trainium2 Bass kernel for nn_Aux2_46969762349381 (scatter_memory).

Computes, for embs [32, 2048, 1024] f32:
  status_probs = softmax(embs @ W_status.T + b_status)   # [B,T,5]
  flight_probs = softmax(embs @ W_flight.T + b_flight)   # [B,T,30]
  out = concat([s0, s2, s1, s4*flight, s3*flight], -1)   # [B,T,63]

Strategy (pure data parallel over batch, 8 cores; full inputs in, full
output out):
  - each core owns 4 batches = 8192 tokens, token t = p*64 + i
    (p = SBUF partition, i = token-tile index) so both the embs loads and
    the out stores are contiguous >=4KB per partition.
  - embs tiles load naturally [128 tok, 1024 emb]; PE transpose (identity
    matmul) flips 128x128 blocks into PSUM; DVE/ACT copy them to SBUF
    giving embsT [128 emb, 8*512 tok].
  - 8 accumulating matmuls (lhsT = host-pretransposed W [128,35] per
    emb-chunk, rhs = embsT chunk [128, 512], float32r) -> psum [35, 512]
    logits.T per 512-token group.
  - ScalarE exp reads the PSUM logits with the per-partition class bias
    fused into the activation -> expT [35, 512] in SBUF.
  - PE transposes expT back to [128 tok, 35] PSUM; DVE does the softmax
    normalization + outer-product scatter into [128, ntile, 63] which DMAs
    out as >=2KB contiguous chunks per partition.

Default precision: embs are cast f32->bf16 during the SWDGE load DMA and
the two tiny matmuls run in bf16 (abs-max relative error ~1.7e-3 vs the
f32 reference; the small heads keep everything else f32). Set
BASS_KERNEL_PRECISE=1 for an all-f32/f32r build (~1.2e-4, ~30% slower:
PE is_transpose runs 2 cycles/row for f32 vs 1 for bf16 and the PE clock
stays at 1.2 GHz because transpose-mode does not engage the HAM).
"""

import os
import sys

import numpy as np

for _p in ("/opt/trn_rl_repo", "/root/.axon_site/_ro/trn_rl_repo"):
    if os.path.isdir(_p) and _p not in sys.path:
        sys.path.insert(0, _p)

from contextlib import ExitStack

import concourse.bass as bass
import concourse.tile as tile
from concourse import mybir
from concourse.bass_utils import run_bass_kernel_spmd

N_CORES = 8
B, T, E = 32, 2048, 1024
NS, NF = 5, 30
NCLS = NS + NF          # 35 combined classes
OUTC = 63
P = 128                 # SBUF partitions
ECH = E // P            # 8 emb chunks of 128
GT = 4                  # token tiles (of 128 tokens) per matmul group
GTOK = GT * P           # 512 tokens per group
AG = 2                  # groups per assembly batch
F32 = mybir.dt.float32
F32R = mybir.dt.float32r
BF16 = mybir.dt.bfloat16
EXP = mybir.ActivationFunctionType.Exp


_CTRL_INSTS = ("InstDrain", "InstNoOp", "InstEventSemaphore",
               "InstUnconditionalBranch", "InstCompareAndBranch", "InstISA")


def _split_multiwait(nc, max_waits=1):
    """Workaround for this walrus build rejecting more than one sem-wait per
    instruction (verified: even 2-wait compute ops fail codegen): move extra
    waits onto single-wait NoOps just before the instruction."""
    for bb in nc.m.functions[0].blocks:
        insts = list(bb.instructions)
        new_list = []
        changed = False
        for inst in insts:
            si = inst.sync_info
            cap = 1 if type(inst).__name__ in _CTRL_INSTS else max_waits
            if si is not None and si.on_wait and len(si.on_wait) > cap:
                waits = list(si.on_wait)
                for w in waits[:-cap]:
                    nop = mybir.InstNoOp(
                        name=nc.get_next_instruction_name(),
                        ins=[],
                        outs=[],
                        engine=inst.engine,
                        sync_info=mybir.SyncInfo(on_wait=[w], on_update=[]),
                    )
                    nc.register_instruction(nop)
                    new_list.append(nop)
                    changed = True
                inst.sync_info = mybir.SyncInfo(
                    on_wait=waits[-cap:], on_update=list(si.on_update)
                )
            new_list.append(inst)
        if changed:
            bb.instructions = new_list


def build_program(tok, copy_split=4, mm_dtype=F32R, tr_dtype=F32, loop_reps=0,
                  passes=1, lpd=2, emb_bufs=4):
    """Build the per-core Bass program for `tok` tokens (tok % 1024 == 0).

    loop_reps > 0 wraps the whole body in a hardware For_i loop executing it
    that many times — benchmarking only (the axon dispatch overhead is ~80ms,
    so single-shot wall timing can't see the ~100us kernel).
    """
    S = tok // P            # token tiles per core
    n_groups = S // GT
    n_batches = n_groups // AG
    NT = AG * GT            # tiles per assembly batch (8)

    nc = bass.Bass("TRN2", num_devices=N_CORES)
    embs_d = nc.dram_tensor("embs", [tok, E], F32, kind="ExternalInput")
    w_d = nc.dram_tensor("wt", [P, ECH * NCLS], F32, kind="ExternalInput")
    b_d = nc.dram_tensor("bias", [NCLS, 1], F32, kind="ExternalInput")
    id_d = nc.dram_tensor("ident", [P, P], F32, kind="ExternalInput")
    out_d = nc.dram_tensor("out", [tok, OUTC], F32, kind="ExternalOutput")

    with tile.TileContext(nc) as tc, ExitStack() as ctx:
        consts = ctx.enter_context(tc.tile_pool(name="consts", bufs=1))
        emb_pool = ctx.enter_context(tc.tile_pool(name="emb", bufs=emb_bufs))
        embT_pool = ctx.enter_context(tc.tile_pool(name="embT", bufs=2))
        expT_pool = ctx.enter_context(tc.tile_pool(name="expT", bufs=2))
        small = ctx.enter_context(tc.tile_pool(name="small", bufs=2))
        outsb = ctx.enter_context(tc.tile_pool(name="outsb", bufs=2))
        psT_pool = ctx.enter_context(tc.tile_pool(name="psT", bufs=4, space="PSUM"))
        psmm_pool = ctx.enter_context(tc.tile_pool(name="psmm", bufs=2, space="PSUM"))
        psxb_pool = ctx.enter_context(tc.tile_pool(name="psxb", bufs=2, space="PSUM"))

        w_raw = consts.tile([P, ECH * NCLS], F32)
        nc.sync.dma_start(w_raw[:], w_d.ap())
        b_sb = consts.tile([NCLS, 1], F32)
        nc.sync.dma_start(b_sb[:], b_d.ap())
        id_sb = consts.tile([P, P], F32)
        if tr_dtype is F32:
            nc.sync.dma_start(id_sb[:], id_d.ap())
        else:
            nc.sync.dma_start(id_sb[:].bitcast(tr_dtype), id_d.ap().bitcast(tr_dtype))
        w_sb = consts.tile([P, ECH * NCLS], mm_dtype)
        if mm_dtype is F32:
            w_sb = w_raw
        else:
            nc.vector.tensor_copy(w_sb[:], w_raw[:])
        if tr_dtype is BF16:
            id_bf = consts.tile([P, P], BF16)
            nc.vector.tensor_copy(id_bf[:], id_sb[:])

        # Trigger the ACT exp table load (~2.7us) immediately so it overlaps
        # the first embs DMAs instead of stalling the first real exp.
        warm = consts.tile([NCLS, 1], F32)
        nc.scalar.activation(warm[:], b_sb[:], EXP)

        embs_v = embs_d.ap().rearrange("(p i) e -> p i e", p=P, i=S)
        out_v = out_d.ap().rearrange("(p i) c -> p i c", p=P, i=S)

        loop_ctx = tc.For_i(0, loop_reps, 1) if loop_reps else None
        if loop_ctx is not None:
            ctx.enter_context(loop_ctx)

        for ab in range(n_batches * passes):
            ab = ab % n_batches
            ps_xb = psxb_pool.tile([P, NT * NCLS], F32)
            for g2 in range(AG):
                g = ab * AG + g2
                embT = embT_pool.tile([P, ECH * GTOK], mm_dtype)
                embT_v = embT[:].rearrange("p (j t) -> p j t", t=GTOK)
                copy_k = 0
                # rows p*64 + 4g + c are adjacent, so each partition reads
                # lpd*E contiguous f32 per DMA at full rate.
                if tr_dtype is BF16:
                    tr = lambda ap: ap
                    ident = id_bf
                    nat_dt = BF16
                else:
                    tr = (lambda ap: ap.bitcast(tr_dtype)) if tr_dtype is not F32 else (lambda ap: ap)
                    ident = id_sb
                    nat_dt = F32
                chunks = []
                for lc in range(GT // lpd):
                    emb_ch = emb_pool.tile([P, lpd * E], nat_dt)
                    dst3 = emb_ch[:].rearrange("p (c e) -> p c e", e=E)
                    src3 = embs_v[:, g * GT + lc * lpd:g * GT + (lc + 1) * lpd, :]
                    if tr_dtype is BF16:
                        nc.gpsimd.dma_start(dst3, src3)  # SWDGE f32->bf16 cast
                    else:
                        nc.sync.dma_start(tr(dst3), tr(src3))
                    chunks.append(dst3)
                for c in range(GT):
                    nat3 = chunks[c // lpd]
                    for half in range(2):
                        psT = psT_pool.tile([P, 4 * P], nat_dt)
                        for jj in range(4):
                            j = half * 4 + jj
                            nc.tensor.matmul(
                                tr(psT[:, jj * P:(jj + 1) * P]),
                                tr(nat3[:, c % lpd, j * P:(j + 1) * P]),
                                tr(ident[:]),
                                is_transpose=True,
                            )
                        dst = embT_v[:, half * 4:(half + 1) * 4, c * P:(c + 1) * P]
                        src = psT[:].rearrange("p (j t) -> p j t", j=4)
                        if copy_k < copy_split:
                            nc.scalar.copy(dst, src)
                        else:
                            nc.vector.tensor_copy(dst, src)
                        copy_k += 1
                ps_mm = psmm_pool.tile([NCLS, GTOK], F32)
                for j in range(ECH):
                    nc.tensor.matmul(
                        ps_mm[:],
                        w_sb[:, j * NCLS:(j + 1) * NCLS],
                        embT[:, j * GTOK:(j + 1) * GTOK],
                        start=(j == 0),
                        stop=(j == ECH - 1),
                    )
                expT = expT_pool.tile([NCLS, GTOK], F32)
                nc.scalar.activation(expT[:], ps_mm[:], EXP, bias=b_sb[:], scale=1.0)
                for c in range(GT):
                    it = g2 * GT + c
                    nc.tensor.matmul(
                        ps_xb[:, it * NCLS:(it + 1) * NCLS],
                        expT[:, c * P:(c + 1) * P],
                        id_sb[0:NCLS, 0:NCLS],
                        is_transpose=True,
                    )

            # ---- assembly for NT tiles (tokens p*64 + ab*NT .. +NT) ----
            X = ps_xb[:].rearrange("p (i c) -> p i c", c=NCLS)  # [128, NT, 35]
            sums = small.tile([P, 2 * NT], F32)
            nc.vector.reduce_sum(sums[:, 0:NT], X[:, :, 0:NS], axis=mybir.AxisListType.X)
            nc.vector.reduce_sum(
                sums[:, NT:2 * NT], X[:, :, NS:NCLS], axis=mybir.AxisListType.X
            )
            inv = small.tile([P, 2 * NT], F32)
            nc.vector.reciprocal(inv[:], sums[:])
            inv_s = inv[:, 0:NT]
            inv_f = inv[:, NT:2 * NT]
            dd = small.tile([P, NT], F32)
            nc.vector.tensor_mul(dd[:], inv_s, inv_f)
            cc = small.tile([P, 2 * NT], F32)
            nc.vector.tensor_mul(cc[:, 0:NT], X[:, :, 4], dd[:])  # book scale
            nc.vector.tensor_mul(cc[:, NT:2 * NT], X[:, :, 3], dd[:])  # change
            o_sb = outsb.tile([P, NT * OUTC], F32)
            O = o_sb[:].rearrange("p (i c) -> p i c", c=OUTC)
            inv_s3 = inv_s.unsqueeze(2)
            nc.vector.tensor_mul(O[:, :, 0:1], X[:, :, 0:1], inv_s3)
            nc.vector.tensor_mul(O[:, :, 1:2], X[:, :, 2:3], inv_s3)
            nc.vector.tensor_mul(O[:, :, 2:3], X[:, :, 1:2], inv_s3)
            nc.vector.tensor_mul(
                O[:, :, 3:3 + NF],
                X[:, :, NS:NCLS],
                cc[:, 0:NT].unsqueeze(2).broadcast_to((P, NT, NF)),
            )
            nc.vector.tensor_mul(
                O[:, :, 3 + NF:OUTC],
                X[:, :, NS:NCLS],
                cc[:, NT:2 * NT].unsqueeze(2).broadcast_to((P, NT, NF)),
            )
            store_eng = nc.sync if tr_dtype is BF16 else nc.gpsimd
            store_eng.dma_start(out_v[:, ab * NT:(ab + 1) * NT, :], o_sb[:].rearrange("p (i c) -> p i c", c=OUTC))

    _split_multiwait(nc)
    return nc


def host_inputs(W_status, b_status, W_flight, b_flight):
    W = np.concatenate([np.asarray(W_status), np.asarray(W_flight)], axis=0)
    W = np.ascontiguousarray(W, dtype=np.float32)          # [35, 1024]
    # w_host[p, j*35 + c] = W[c, j*128 + p]
    w_host = np.ascontiguousarray(
        W.T.reshape(ECH, P, NCLS).transpose(1, 0, 2).reshape(P, ECH * NCLS)
    )
    b_host = np.ascontiguousarray(
        np.concatenate([np.asarray(b_status), np.asarray(b_flight)]).reshape(NCLS, 1),
        dtype=np.float32,
    )
    ident = np.eye(P, dtype=np.float32)
    return w_host, b_host, ident


_program_cache = {}


def kernel(embs, W_status, b_status, W_flight, b_flight, **run_kwargs):
    embs = np.ascontiguousarray(np.asarray(embs), dtype=np.float32)
    tok = embs.shape[0] * embs.shape[1] // N_CORES
    w_host, b_host, ident = host_inputs(W_status, b_status, W_flight, b_flight)

    precise = bool(os.environ.get("BASS_KERNEL_PRECISE"))
    key = (tok, precise)
    nc = _program_cache.get(key)
    if nc is None:
        if precise:
            # f32r everywhere: rel err ~1.2e-4, ~30% slower (PE transposes)
            nc = build_program(tok, tr_dtype=F32R, lpd=1)
        else:
            # bf16 embs (cast during DMA) + bf16 PE: rel err ~1.7e-3
            nc = build_program(tok, tr_dtype=BF16, mm_dtype=BF16)
        _program_cache[key] = nc

    embs_flat = embs.reshape(-1, E)
    in_maps = [
        {
            "embs": embs_flat[c * tok:(c + 1) * tok],
            "wt": w_host,
            "bias": b_host,
            "ident": ident,
        }
        for c in range(N_CORES)
    ]
    res = run_bass_kernel_spmd(
        nc, in_maps, core_ids=list(range(N_CORES)), **run_kwargs
    )
    out = np.concatenate([res.results[c]["out"] for c in range(N_CORES)], axis=0)
    out = out.reshape(embs.shape[0], embs.shape[1], OUTC)
    if run_kwargs:
        return out, res
    return out
